# revision 49
# baseline (speedup 1.0000x reference)
"""fp8 quantized matmul y = fp8(x) @ fp8(W)^T on 8 Trainium2 NeuronCores.

Reference semantics: x[M,K] f32 and W[N,K] f32 are each cast to
float8_e4m3fn (OCP, round-to-nearest-even) and the matmul accumulates in
fp32.  The cast is a pure element-wise dtype conversion, done here on the
host with ml_dtypes (bit-identical to the reference's jax cast for the
value range involved: |x| < 16, |W| <= 2^-6, both far below 240 where the
OCP and IEEE e4m3 encodings coincide).

Sharding: 2D (4 x 2) — core c computes the [2048, 2048] block
y[(c//2)*2048 :, (c%2)*2048 :] from a 2048-row x shard (8 MiB fp8) and a
2048-col W shard (8 MiB fp8), both SBUF-resident.  16 MiB/core minimizes
per-core and aggregate HBM traffic (vs 20 for 8x1 data-parallel), which
matters because all 8 cores pull their inputs concurrently at ramp.

Device kernel: fp8 DoubleRow matmuls.  The PE streams back-to-back at the
fp8 peak (512+6 cycles per [128,512]-out matmul, LDWEIGHTS pipelined
under the previous stream), so the kernel is compute-roofline: 1024 MMs
x ~216 ns = 221 us of PE stream.  Everything else is ramp/tail
engineering:

 * Ramp phases A/B are K-SLICED: all 8 PSUM banks hold one m-tile each
   and the kernel sweeps k-pair groups across them.  Consumption of
   fresh bytes is then fine-grained (~384 KiB per 1.73 us, split across
   both HWDGE rings) and tracks the rings' ramping delivery rate from
   ~10 us on — a tile-major order would instead stall ~8 us on the
   2.5 MiB (x tile + W column) lump the first output tile needs.
 * x rides mostly the SP ring as 32 k-sliced half-tiles (256 KiB,
   2 KiB/line; the first one further split so the first matmul waits on
   64-128 KiB); W column 0 rides the otherwise-idle ACT ring as 9
   k-groups — the two operands' first slivers land concurrently, not
   FIFO-serialized.  Each compute phase's x demand is balanced across
   both rings (every 3rd phase-A group and every even phase-B group on
   ACT, interleaved in consumption order).
 * 20 tiny N=128 warmup matmuls on one memset tile bridge the startup
   barrier -> first-data window and release the HAM clock gate / PE
   p-state ramp before real matmuls begin (HW needs ~4 us of PE busy
   to reach the full clock).
 * Phase C (columns 1..3) is tile-major; whole 2 MiB W columns land
   with a ~5x margin over their 55 us consumption period.
 * DVE evacuates PSUM -> SBUF as fp16 (halving store traffic; ~5e-4
   relative noise, far under the 2e-2 gate); stores ride the ACT ring;
   the last two tiles are evicted in halves so the final DVE copy and
   store pipeline into the kernel-tail drain (the very last store rides
   the by-then-idle SP ring).

Host-side layouts are pre-transposed so every DMA is a large contiguous
per-partition transfer:
  xk[g, h, p, t, j] = fp8(x_shard)[h*1024 + j, (2g+t)*128 + p]
  wt[nt, p, kt, n]  = fp8(W_shard)[nt*512 + n, kt*128 + p]
The [p, ..., cols] SBUF tiles feed nc.tensor.matmul with the DoubleRow
contraction pair (t, p) -> k = (2g+t)*128 + p over two consecutive
k-subtiles, identically on both operands.
"""

import numpy as np
import ml_dtypes

P = 128          # partitions
N_CORES = 8
GM, GN = 4, 2              # core grid: 4-way over M, 2-way over N
M, K, N = 8192, 4096, 4096
MC = M // GM               # 2048 rows of x per core
NC = N // GN               # 2048 out-features per core
MT = MC // P               # 16 m-tiles per core
KT = K // P                # 32 k-subtiles
NP = KT // 2               # 16 k-pair groups (DoubleRow contraction pairs)
NB = 512                   # psum bank width (f32)
NT = NC // NB              # 4 n-tiles per core
MH = MT // 2               # 8 m-tiles per k-sliced half (one PSUM bank each)

_NC_CACHE = {}

N_WARMUP = 20    # tiny N=128 PE matmuls bridging startup barrier -> stream
# kt-subtiles per w column-0 k-group: first two groups are single
# DoubleRow pairs (128 KiB) so the very first matmuls' dependencies land
# as early as possible; the rest are 2-pair groups (256 KiB, 2 KiB lines)
W0_KQS = (2, 2, 4, 4, 4, 4, 4, 4, 4)


def _emit(nc, tc, mybir, X, W, Y, mt_n, nt_n, kt_n, nb):
    fp8 = mybir.dt.float8e4
    f32 = mybir.dt.float32
    f16 = mybir.dt.float16
    import contextlib

    n_pairs = kt_n // 2
    mh = mt_n // 2
    assert sum(W0_KQS) == kt_n and all(k % 2 == 0 for k in W0_KQS)
    w0_pair_start = []
    acc = 0
    for k in W0_KQS:
        w0_pair_start.append(acc)
        acc += k // 2

    with contextlib.ExitStack() as ctx:
        warm = ctx.enter_context(tc.tile_pool(name="warm", bufs=1))
        xpool = ctx.enter_context(tc.tile_pool(name="xpool", bufs=1))
        wpool = ctx.enter_context(tc.tile_pool(name="wpool", bufs=1))
        spool = ctx.enter_context(tc.tile_pool(name="spool", bufs=8))
        ppool = ctx.enter_context(
            tc.tile_pool(name="ppool", bufs=8, space="PSUM")
        )

        # PE warmup on one tiny memset tile (used as both operands):
        # occupies the tensor engine from the end of the startup barrier
        # until the first input DMAs land, so the HAM clock gate is
        # released and the p-state ramped before real matmuls begin.
        # The memset rides the vector engine (free earliest after the
        # prologue).
        wm = warm.tile([P, 2, P], fp8, name="wm", tag="wm")
        nc.vector.memset(wm, 0.0)
        wm_ps = ppool.tile([P, P], f32, name="wm_ps", tag="ps")
        for _ in range(N_WARMUP):
            nc.tensor.matmul(
                wm_ps,
                wm,
                wm,
                start=True,
                stop=True,
                perf_mode=mybir.MatmulPerfMode.DoubleRow,
            )

        # Input loads.  x k-sliced half-tiles ride the SP ring; W column
        # 0's k-groups (and wt1 + all output stores) ride the ACT ring,
        # so the two operands' first slivers transfer concurrently
        # (HWDGE rings are FIFO per engine).  wt2/wt3 follow x on SP.
        xk = [[None] * 2 for _ in range(n_pairs)]   # [g][h] -> [P,2,1024]
        xk00_plan = [(0, 2 * P), (1, 2 * P), (2, 4 * P)]  # pair-0 h=0 splits
        xk00 = [None] * len(xk00_plan)
        wt = [None] * nt_n                          # whole W cols (nt>=1)
        w0g = [None] * len(W0_KQS)                  # W col-0 k-groups

        def load_xk(g, h, engine=None):
            t = xpool.tile(
                [P, 2, mh * P], fp8, name=f"xk{g}_{h}", tag=f"xk{g}_{h}"
            )
            (engine or nc.sync).dma_start(out=t, in_=X[g, h, :, :, :])
            xk[g][h] = t

        def load_xk00(q, width):
            # pair 0 of the h=0 half, split in [P,2,width] tiles so the
            # stream's very first matmul only waits on the first of them
            j0 = sum(w for _, w in xk00_plan[:q])
            t = xpool.tile(
                [P, 2, width], fp8, name=f"xk00{q}", tag=f"xk00{q}"
            )
            nc.sync.dma_start(out=t, in_=X[0, 0, :, :, j0 : j0 + width])
            xk00[q] = (j0, width, t)

        def load_w(nt, engine):
            t = wpool.tile([P, kt_n, nb], fp8, name=f"wt{nt}", tag=f"wt{nt}")
            engine.dma_start(out=t, in_=W[nt, :, :, :])
            wt[nt] = t

        def load_w0(g):
            kq = W0_KQS[g]
            k0 = 2 * w0_pair_start[g]
            t = wpool.tile([P, kq, nb], fp8, name=f"w0g{g}", tag=f"w0g{g}")
            nc.scalar.dma_start(out=t, in_=W[0, :, k0 : k0 + kq, :])
            w0g[g] = t

        # Phase-A delivery is ring-balanced: the SP ring alone would need
        # ~148 GB/s for x while ACT carries only ~74 for w0 — so every
        # third phase-A x group rides ACT instead, interleaved with the
        # w0 groups in consumption order (HWDGE rings are FIFO).
        act_xg = set(range(2, n_pairs, 3))
        w0_emitted = 0

        def emit_w0_through(pair):
            nonlocal w0_emitted
            while (
                w0_emitted < len(W0_KQS)
                and w0_pair_start[w0_emitted] <= pair
            ):
                load_w0(w0_emitted)
                w0_emitted += 1

        load_w0(0)
        w0_emitted = 1
        for q, (_, width) in enumerate(xk00_plan):
            load_xk00(q, width)
        for g in range(1, n_pairs):
            if g in act_xg:
                emit_w0_through(g - 1)
                load_xk(g, 0, nc.scalar)
            else:
                load_xk(g, 0)
        emit_w0_through(n_pairs - 1)
        # phase-B x groups are ring-balanced too (evens on ACT ahead of
        # wt1 — ACT only carries stores by then; odds on SP) so phase B
        # doesn't stall on a single ring's FIFO
        for g in range(n_pairs):
            load_xk(g, 1, nc.scalar if g % 2 == 0 else None)
        load_w(1, nc.scalar)
        load_w(2, nc.sync)
        load_w(3, nc.sync)

        def x_slice(mt, t2):
            h, mi = divmod(mt, mh)
            if t2 == 0 and h == 0:
                for j0, width, t in xk00:
                    off = mi * P - j0
                    if 0 <= off < width:
                        return t[:, :, off : off + P]
            return xk[t2][h][:, :, mi * P : (mi + 1) * P]

        def w_slice(nt, t2):
            if nt == 0:
                g = len(W0_KQS) - 1
                while w0_pair_start[g] > t2:
                    g -= 1
                l = t2 - w0_pair_start[g]
                return w0g[g][:, 2 * l : 2 * l + 2, :]
            return wt[nt][:, 2 * t2 : 2 * t2 + 2, :]

        def evict(ps, nt, mt, n_off, n_len, engine=None):
            st = spool.tile([P, n_len], f16, name="st", tag="st")
            nc.vector.tensor_copy(out=st, in_=ps)
            # Y is tile-blocked [nt, mt, P, nb] so a whole-tile store is
            # per-partition contiguous in DRAM — short descriptor chains
            # keep the issue and the final store's completion latency low
            (engine or nc.scalar).dma_start(
                out=Y[nt, mt, :, n_off : n_off + n_len],
                in_=st,
            )

        # Phases A (mt 0..7) and B (mt 8..15): k-sliced sweep of column
        # 0 with all 8 PSUM banks resident.
        for h in range(2):
            ps = [
                ppool.tile([P, nb], f32, name="ps", tag="ps")
                for _ in range(mh)
            ]
            for t2 in range(n_pairs):
                for mi in range(mh):
                    nc.tensor.matmul(
                        ps[mi],
                        x_slice(h * mh + mi, t2),
                        w_slice(0, t2),
                        start=(t2 == 0),
                        stop=(t2 == n_pairs - 1),
                        perf_mode=mybir.MatmulPerfMode.DoubleRow,
                    )
            for mi in range(mh):
                evict(ps[mi], 0, h * mh + mi, 0, nb)

        # Phase C: columns 1..3, tile-major.
        def emit_tile(nt, mt, n_off, n_len, engine=None):
            ps = ppool.tile([P, n_len], f32, name="ps", tag="ps")
            for t2 in range(n_pairs):
                nc.tensor.matmul(
                    ps,
                    x_slice(mt, t2),
                    w_slice(nt, t2)[:, :, n_off : n_off + n_len],
                    start=(t2 == 0),
                    stop=(t2 == n_pairs - 1),
                    perf_mode=mybir.MatmulPerfMode.DoubleRow,
                )
            evict(ps, nt, mt, n_off, n_len, engine)

        for nt in range(1, nt_n):
            for mt in range(mt_n):
                last = nt == nt_n - 1 and mt == mt_n - 1
                second_last = nt == nt_n - 1 and mt == mt_n - 2
                if last:
                    # halve the very last output tile so its first
                    # half's PSUM eviction + store overlap the second
                    # half's matmuls instead of sitting exposed before
                    # the kernel-tail drain (quarters would cost ~1.5 us
                    # of N=128 dispatch-floor overhead on the PE); the
                    # final store rides the by-then-idle SP ring so it
                    # doesn't queue behind the earlier stores on ACT
                    emit_tile(nt, mt, 0, nb // 2)
                    emit_tile(nt, mt, nb // 2, nb - nb // 2, nc.sync)
                elif second_last:
                    emit_tile(nt, mt, 0, nb // 2)
                    emit_tile(nt, mt, nb // 2, nb - nb // 2)
                else:
                    emit_tile(nt, mt, 0, nb)


def _build(mt_n=MT, nt_n=NT, kt_n=KT, nb=NB, hw=True):
    import concourse.bacc as bacc
    import concourse.mybir as mybir
    import concourse.tile as tile
    from concourse.bass_interp import get_hw_module

    nc = bacc.Bacc("TRN2", target_bir_lowering=False, debug=False)
    X = nc.dram_tensor(
        "xk",
        [kt_n // 2, 2, P, 2, (mt_n // 2) * P],
        mybir.dt.float8e4,
        kind="ExternalInput",
    ).ap()
    W = nc.dram_tensor(
        "wt", [nt_n, P, kt_n, nb], mybir.dt.float8e4, kind="ExternalInput"
    ).ap()
    Y = nc.dram_tensor(
        "y", [nt_n, mt_n, P, nb], mybir.dt.float16, kind="ExternalOutput"
    ).ap()
    with tile.TileContext(nc) as tc:
        _emit(nc, tc, mybir, X, W, Y, mt_n, nt_n, kt_n, nb)
    nc.compile()
    if hw:
        nc.m = get_hw_module(nc.m)
    return nc


def _get_nc():
    if "nc" not in _NC_CACHE:
        _NC_CACHE["nc"] = _build()
    return _NC_CACHE["nc"]


def _quantize(a):
    # OCP e4m3fn RNE cast (matches jax astype), then reinterpret as the
    # IEEE e4m3 dtype the BIR tensor declares (identical bits below 240).
    return a.astype(ml_dtypes.float8_e4m3fn).view(ml_dtypes.float8_e4m3)


def _in_maps(x, W):
    xq = _quantize(np.ascontiguousarray(x))
    wq = _quantize(np.ascontiguousarray(W))
    # per N-shard: wt[nt, p, kt, n] = w_shard[nt*NB + n, kt*P + p]
    wts = []
    for j in range(GN):
        ws = wq[j * NC : (j + 1) * NC]
        wts.append(
            np.ascontiguousarray(ws.reshape(NT, NB, KT, P).transpose(0, 3, 2, 1))
        )
    xts = []
    for i in range(GM):
        xc = xq[i * MC : (i + 1) * MC]
        # xk[g, h, p, t, j] = xc[h*(MH*P) + j, (2g+t)*128 + p]
        v = xc.reshape(2, MH * P, NP, 2, P).transpose(2, 0, 4, 3, 1)
        xts.append(np.ascontiguousarray(v))
    return [{"xk": xts[c // GN], "wt": wts[c % GN]} for c in range(N_CORES)]


def _ensure_axon_ntff_hook():
    # Under axon, run_bass_kernel_spmd(trace=True) imports
    # antenv.axon_hooks, which some images lack even though the boot
    # machinery that implements the hook is present.  Register a shim so
    # tracing degrades gracefully instead of raising.
    import sys

    if "antenv.axon_hooks" in sys.modules:
        return
    try:
        from concourse._compat import axon_active

        if not axon_active():
            return
        import importlib.util

        if importlib.util.find_spec("antenv.axon_hooks") is not None:
            return
        import types

        import antenv

        hook = None
        try:
            import trn_agent_boot.trn_boot as _tb

            hook = _tb._ntff_profile_via_ctypes("/opt/axon/libaxon_pjrt.so")
        except Exception:
            hook = None
        mod = types.ModuleType("antenv.axon_hooks")
        mod._hook = hook
        mod.get_axon_ntff_profile_hook = lambda: mod._hook
        def _set(h):
            mod._hook = h
        mod.set_axon_ntff_profile_hook = _set
        antenv.axon_hooks = mod
        sys.modules["antenv.axon_hooks"] = mod
    except Exception:
        pass


def _run(in_maps, trace=False):
    from concourse.bass_utils import run_bass_kernel_spmd

    _ensure_axon_ntff_hook()
    nc = _get_nc()
    return run_bass_kernel_spmd(
        nc, in_maps, core_ids=list(range(len(in_maps))), trace=trace
    )


def _unblock(y):
    # y: [NT, MT, P, NB] f16 tile-blocked -> [MC, NC]
    return y.transpose(1, 2, 0, 3).reshape(MC, NC)


def kernel(x, W):
    res = _run(_in_maps(x, W))
    rows = [
        np.concatenate(
            [_unblock(res.results[i * GN + j]["y"]) for j in range(GN)],
            axis=1,
        )
        for i in range(GM)
    ]
    return np.concatenate(rows, axis=0).astype(np.float32, copy=False)


# revision 50
# speedup vs baseline: 1.0084x; 1.0084x over previous
"""fp8 quantized matmul y = fp8(x) @ fp8(W)^T on 8 Trainium2 NeuronCores.

Reference semantics: x[M,K] f32 and W[N,K] f32 are each cast to
float8_e4m3fn (OCP, round-to-nearest-even) and the matmul accumulates in
fp32.  The cast is a pure element-wise dtype conversion, done here on the
host with ml_dtypes (bit-identical to the reference's jax cast for the
value range involved: |x| < 16, |W| <= 2^-6, both far below 240 where the
OCP and IEEE e4m3 encodings coincide).

Sharding: 2D (4 x 2) — core c computes the [2048, 2048] block
y[(c//2)*2048 :, (c%2)*2048 :] from a 2048-row x shard (8 MiB fp8) and a
2048-col W shard (8 MiB fp8), both SBUF-resident.  16 MiB/core minimizes
per-core and aggregate HBM traffic (vs 20 for 8x1 data-parallel), which
matters because all 8 cores pull their inputs concurrently at ramp.

Device kernel: fp8 DoubleRow matmuls.  The PE streams back-to-back at the
fp8 peak (512+6 cycles per [128,512]-out matmul, LDWEIGHTS pipelined
under the previous stream), so the kernel is compute-roofline: 1024 MMs
x ~216 ns = 221 us of PE stream.  Everything else is ramp/tail
engineering:

 * Ramp phases A/B are K-SLICED: all 8 PSUM banks hold one m-tile each
   and the kernel sweeps k-pair groups across them.  Consumption of
   fresh bytes is then fine-grained (~384 KiB per 1.73 us, split across
   both HWDGE rings) and tracks the rings' ramping delivery rate from
   ~10 us on — a tile-major order would instead stall ~8 us on the
   2.5 MiB (x tile + W column) lump the first output tile needs.
 * x rides mostly the SP ring as 32 k-sliced half-tiles (256 KiB,
   2 KiB/line; the first one further split so the first matmul waits on
   64-128 KiB); W column 0 rides the otherwise-idle ACT ring as 9
   k-groups — the two operands' first slivers land concurrently, not
   FIFO-serialized.  Each compute phase's x demand is balanced across
   both rings (every 3rd phase-A group and every even phase-B group on
   ACT, interleaved in consumption order).
 * 20 tiny N=128 warmup matmuls on one memset tile bridge the startup
   barrier -> first-data window and release the HAM clock gate / PE
   p-state ramp before real matmuls begin (HW needs ~4 us of PE busy
   to reach the full clock).
 * Phase C (columns 1..3) is tile-major; whole 2 MiB W columns land
   with a ~5x margin over their 55 us consumption period.
 * DVE evacuates PSUM -> SBUF as fp16 (halving store traffic; ~5e-4
   relative noise, far under the 2e-2 gate); stores ride the ACT ring;
   the last two tiles are evicted in halves so the final DVE copy and
   store pipeline into the kernel-tail drain (the very last store rides
   the by-then-idle SP ring).

Host-side layouts are pre-transposed so every DMA is a large contiguous
per-partition transfer:
  xk[g, h, p, t, j] = fp8(x_shard)[h*1024 + j, (2g+t)*128 + p]
  wt[nt, p, kt, n]  = fp8(W_shard)[nt*512 + n, kt*128 + p]
The [p, ..., cols] SBUF tiles feed nc.tensor.matmul with the DoubleRow
contraction pair (t, p) -> k = (2g+t)*128 + p over two consecutive
k-subtiles, identically on both operands.
"""

import numpy as np
import ml_dtypes

P = 128          # partitions
N_CORES = 8
GM, GN = 4, 2              # core grid: 4-way over M, 2-way over N
M, K, N = 8192, 4096, 4096
MC = M // GM               # 2048 rows of x per core
NC = N // GN               # 2048 out-features per core
MT = MC // P               # 16 m-tiles per core
KT = K // P                # 32 k-subtiles
NP = KT // 2               # 16 k-pair groups (DoubleRow contraction pairs)
NB = 512                   # psum bank width (f32)
NT = NC // NB              # 4 n-tiles per core
MH = MT // 2               # 8 m-tiles per k-sliced half (one PSUM bank each)

_NC_CACHE = {}

N_WARMUP = 23    # tiny N=128 PE matmuls bridging startup barrier -> stream
# kt-subtiles per w column-0 k-group: first two groups are single
# DoubleRow pairs (128 KiB) so the very first matmuls' dependencies land
# as early as possible; the rest are 2-pair groups (256 KiB, 2 KiB lines)
W0_KQS = (2, 2, 4, 4, 4, 4, 4, 4, 4)


def _emit(nc, tc, mybir, X, W, Y, mt_n, nt_n, kt_n, nb):
    fp8 = mybir.dt.float8e4
    f32 = mybir.dt.float32
    f16 = mybir.dt.float16
    import contextlib

    n_pairs = kt_n // 2
    mh = mt_n // 2
    assert sum(W0_KQS) == kt_n and all(k % 2 == 0 for k in W0_KQS)
    w0_pair_start = []
    acc = 0
    for k in W0_KQS:
        w0_pair_start.append(acc)
        acc += k // 2

    with contextlib.ExitStack() as ctx:
        warm = ctx.enter_context(tc.tile_pool(name="warm", bufs=1))
        xpool = ctx.enter_context(tc.tile_pool(name="xpool", bufs=1))
        wpool = ctx.enter_context(tc.tile_pool(name="wpool", bufs=1))
        spool = ctx.enter_context(tc.tile_pool(name="spool", bufs=8))
        ppool = ctx.enter_context(
            tc.tile_pool(name="ppool", bufs=8, space="PSUM")
        )

        # PE warmup on one tiny memset tile (used as both operands):
        # occupies the tensor engine from the end of the startup barrier
        # until the first input DMAs land, so the HAM clock gate is
        # released and the p-state ramped before real matmuls begin.
        # The memset rides the vector engine (free earliest after the
        # prologue).
        wm = warm.tile([P, 2, P], fp8, name="wm", tag="wm")
        nc.vector.memset(wm, 0.0)
        wm_ps = ppool.tile([P, P], f32, name="wm_ps", tag="ps")
        for _ in range(N_WARMUP):
            nc.tensor.matmul(
                wm_ps,
                wm,
                wm,
                start=True,
                stop=True,
                perf_mode=mybir.MatmulPerfMode.DoubleRow,
            )

        # Input loads.  x k-sliced half-tiles ride the SP ring; W column
        # 0's k-groups (and wt1 + all output stores) ride the ACT ring,
        # so the two operands' first slivers transfer concurrently
        # (HWDGE rings are FIFO per engine).  wt2/wt3 follow x on SP.
        xk = [[None] * 2 for _ in range(n_pairs)]   # [g][h] -> [P,2,1024]
        xk00_plan = [(0, 2 * P), (1, 2 * P), (2, 4 * P)]  # pair-0 h=0 splits
        xk00 = [None] * len(xk00_plan)
        wt = [None] * nt_n                          # whole W cols (nt>=1)
        w0g = [None] * len(W0_KQS)                  # W col-0 k-groups

        def load_xk(g, h, engine=None):
            t = xpool.tile(
                [P, 2, mh * P], fp8, name=f"xk{g}_{h}", tag=f"xk{g}_{h}"
            )
            (engine or nc.sync).dma_start(out=t, in_=X[g, h, :, :, :])
            xk[g][h] = t

        def load_xk00(q, width):
            # pair 0 of the h=0 half, split in [P,2,width] tiles so the
            # stream's very first matmul only waits on the first of them
            j0 = sum(w for _, w in xk00_plan[:q])
            t = xpool.tile(
                [P, 2, width], fp8, name=f"xk00{q}", tag=f"xk00{q}"
            )
            nc.sync.dma_start(out=t, in_=X[0, 0, :, :, j0 : j0 + width])
            xk00[q] = (j0, width, t)

        def load_w(nt, engine):
            t = wpool.tile([P, kt_n, nb], fp8, name=f"wt{nt}", tag=f"wt{nt}")
            engine.dma_start(out=t, in_=W[nt, :, :, :])
            wt[nt] = t

        def load_w0(g):
            kq = W0_KQS[g]
            k0 = 2 * w0_pair_start[g]
            t = wpool.tile([P, kq, nb], fp8, name=f"w0g{g}", tag=f"w0g{g}")
            nc.scalar.dma_start(out=t, in_=W[0, :, k0 : k0 + kq, :])
            w0g[g] = t

        # Phase-A delivery is ring-balanced: the SP ring alone would need
        # ~148 GB/s for x while ACT carries only ~74 for w0 — so every
        # third phase-A x group rides ACT instead, interleaved with the
        # w0 groups in consumption order (HWDGE rings are FIFO).
        act_xg = set(range(2, n_pairs, 3))
        w0_emitted = 0

        def emit_w0_through(pair):
            nonlocal w0_emitted
            while (
                w0_emitted < len(W0_KQS)
                and w0_pair_start[w0_emitted] <= pair
            ):
                load_w0(w0_emitted)
                w0_emitted += 1

        load_w0(0)
        w0_emitted = 1
        for q, (_, width) in enumerate(xk00_plan):
            load_xk00(q, width)
        for g in range(1, n_pairs):
            if g in act_xg:
                emit_w0_through(g - 1)
                load_xk(g, 0, nc.scalar)
            else:
                load_xk(g, 0)
        emit_w0_through(n_pairs - 1)
        # phase-B x groups are ring-balanced too (evens on ACT ahead of
        # wt1 — ACT only carries stores by then; odds on SP) so phase B
        # doesn't stall on a single ring's FIFO
        for g in range(n_pairs):
            load_xk(g, 1, nc.scalar if g % 2 == 0 else None)
        load_w(1, nc.scalar)
        load_w(2, nc.sync)
        load_w(3, nc.sync)

        def x_slice(mt, t2):
            h, mi = divmod(mt, mh)
            if t2 == 0 and h == 0:
                for j0, width, t in xk00:
                    off = mi * P - j0
                    if 0 <= off < width:
                        return t[:, :, off : off + P]
            return xk[t2][h][:, :, mi * P : (mi + 1) * P]

        def w_slice(nt, t2):
            if nt == 0:
                g = len(W0_KQS) - 1
                while w0_pair_start[g] > t2:
                    g -= 1
                l = t2 - w0_pair_start[g]
                return w0g[g][:, 2 * l : 2 * l + 2, :]
            return wt[nt][:, 2 * t2 : 2 * t2 + 2, :]

        def evict(ps, nt, mt, n_off, n_len, engine=None):
            st = spool.tile([P, n_len], f16, name="st", tag="st")
            nc.vector.tensor_copy(out=st, in_=ps)
            # Y is tile-blocked [nt, mt, P, nb] so a whole-tile store is
            # per-partition contiguous in DRAM — short descriptor chains
            # keep the issue and the final store's completion latency low
            (engine or nc.scalar).dma_start(
                out=Y[nt, mt, :, n_off : n_off + n_len],
                in_=st,
            )

        # Phases A (mt 0..7) and B (mt 8..15): k-sliced sweep of column
        # 0 with all 8 PSUM banks resident.
        for h in range(2):
            ps = [
                ppool.tile([P, nb], f32, name="ps", tag="ps")
                for _ in range(mh)
            ]
            for t2 in range(n_pairs):
                for mi in range(mh):
                    nc.tensor.matmul(
                        ps[mi],
                        x_slice(h * mh + mi, t2),
                        w_slice(0, t2),
                        start=(t2 == 0),
                        stop=(t2 == n_pairs - 1),
                        perf_mode=mybir.MatmulPerfMode.DoubleRow,
                    )
            for mi in range(mh):
                evict(ps[mi], 0, h * mh + mi, 0, nb)

        # Phase C: columns 1..3, tile-major.
        def emit_tile(nt, mt, n_off, n_len, engine=None):
            ps = ppool.tile([P, n_len], f32, name="ps", tag="ps")
            for t2 in range(n_pairs):
                nc.tensor.matmul(
                    ps,
                    x_slice(mt, t2),
                    w_slice(nt, t2)[:, :, n_off : n_off + n_len],
                    start=(t2 == 0),
                    stop=(t2 == n_pairs - 1),
                    perf_mode=mybir.MatmulPerfMode.DoubleRow,
                )
            evict(ps, nt, mt, n_off, n_len, engine)

        for nt in range(1, nt_n):
            for mt in range(mt_n):
                last = nt == nt_n - 1 and mt == mt_n - 1
                second_last = nt == nt_n - 1 and mt == mt_n - 2
                if last:
                    # halve the very last output tile so its first
                    # half's PSUM eviction + store overlap the second
                    # half's matmuls instead of sitting exposed before
                    # the kernel-tail drain (quarters would cost ~1.5 us
                    # of N=128 dispatch-floor overhead on the PE); the
                    # final store rides the by-then-idle SP ring so it
                    # doesn't queue behind the earlier stores on ACT
                    emit_tile(nt, mt, 0, nb // 2)
                    emit_tile(nt, mt, nb // 2, nb - nb // 2, nc.sync)
                elif second_last:
                    emit_tile(nt, mt, 0, nb // 2)
                    emit_tile(nt, mt, nb // 2, nb - nb // 2)
                else:
                    emit_tile(nt, mt, 0, nb)


def _build(mt_n=MT, nt_n=NT, kt_n=KT, nb=NB, hw=True):
    import concourse.bacc as bacc
    import concourse.mybir as mybir
    import concourse.tile as tile
    from concourse.bass_interp import get_hw_module

    nc = bacc.Bacc("TRN2", target_bir_lowering=False, debug=False)
    X = nc.dram_tensor(
        "xk",
        [kt_n // 2, 2, P, 2, (mt_n // 2) * P],
        mybir.dt.float8e4,
        kind="ExternalInput",
    ).ap()
    W = nc.dram_tensor(
        "wt", [nt_n, P, kt_n, nb], mybir.dt.float8e4, kind="ExternalInput"
    ).ap()
    Y = nc.dram_tensor(
        "y", [nt_n, mt_n, P, nb], mybir.dt.float16, kind="ExternalOutput"
    ).ap()
    with tile.TileContext(nc) as tc:
        _emit(nc, tc, mybir, X, W, Y, mt_n, nt_n, kt_n, nb)
    nc.compile()
    if hw:
        nc.m = get_hw_module(nc.m)
    return nc


def _get_nc():
    if "nc" not in _NC_CACHE:
        _NC_CACHE["nc"] = _build()
    return _NC_CACHE["nc"]


def _quantize(a):
    # OCP e4m3fn RNE cast (matches jax astype), then reinterpret as the
    # IEEE e4m3 dtype the BIR tensor declares (identical bits below 240).
    return a.astype(ml_dtypes.float8_e4m3fn).view(ml_dtypes.float8_e4m3)


def _in_maps(x, W):
    xq = _quantize(np.ascontiguousarray(x))
    wq = _quantize(np.ascontiguousarray(W))
    # per N-shard: wt[nt, p, kt, n] = w_shard[nt*NB + n, kt*P + p]
    wts = []
    for j in range(GN):
        ws = wq[j * NC : (j + 1) * NC]
        wts.append(
            np.ascontiguousarray(ws.reshape(NT, NB, KT, P).transpose(0, 3, 2, 1))
        )
    xts = []
    for i in range(GM):
        xc = xq[i * MC : (i + 1) * MC]
        # xk[g, h, p, t, j] = xc[h*(MH*P) + j, (2g+t)*128 + p]
        v = xc.reshape(2, MH * P, NP, 2, P).transpose(2, 0, 4, 3, 1)
        xts.append(np.ascontiguousarray(v))
    return [{"xk": xts[c // GN], "wt": wts[c % GN]} for c in range(N_CORES)]


def _ensure_axon_ntff_hook():
    # Under axon, run_bass_kernel_spmd(trace=True) imports
    # antenv.axon_hooks, which some images lack even though the boot
    # machinery that implements the hook is present.  Register a shim so
    # tracing degrades gracefully instead of raising.
    import sys

    if "antenv.axon_hooks" in sys.modules:
        return
    try:
        from concourse._compat import axon_active

        if not axon_active():
            return
        import importlib.util

        if importlib.util.find_spec("antenv.axon_hooks") is not None:
            return
        import types

        import antenv

        hook = None
        try:
            import trn_agent_boot.trn_boot as _tb

            hook = _tb._ntff_profile_via_ctypes("/opt/axon/libaxon_pjrt.so")
        except Exception:
            hook = None
        mod = types.ModuleType("antenv.axon_hooks")
        mod._hook = hook
        mod.get_axon_ntff_profile_hook = lambda: mod._hook
        def _set(h):
            mod._hook = h
        mod.set_axon_ntff_profile_hook = _set
        antenv.axon_hooks = mod
        sys.modules["antenv.axon_hooks"] = mod
    except Exception:
        pass


def _run(in_maps, trace=False):
    from concourse.bass_utils import run_bass_kernel_spmd

    _ensure_axon_ntff_hook()
    nc = _get_nc()
    return run_bass_kernel_spmd(
        nc, in_maps, core_ids=list(range(len(in_maps))), trace=trace
    )


def _unblock(y):
    # y: [NT, MT, P, NB] f16 tile-blocked -> [MC, NC]
    return y.transpose(1, 2, 0, 3).reshape(MC, NC)


def kernel(x, W):
    res = _run(_in_maps(x, W))
    rows = [
        np.concatenate(
            [_unblock(res.results[i * GN + j]["y"]) for j in range(GN)],
            axis=1,
        )
        for i in range(GM)
    ]
    return np.concatenate(rows, axis=0).astype(np.float32, copy=False)
